# revision 10
# baseline (speedup 1.0000x reference)
"""CubeQueryAndGroup Trainium2 kernel.

Problem: B=4 batches x (Nb=16384 points, Mb=4096 queries), C=64 feats,
cube radius 0.1 per axis, nsample=32 (first 32 in-cube points by index).

Sharding: 8 cores; core k handles batch k//2, query half k%2 (2048 queries),
with the full batch's 16384 points + features. No cross-core communication.

Per-core device algorithm:
  Phase 1 (mask+score): for each point-chunk (2048 pts broadcast across
    partitions) x query-tile (128 queries on partitions):
      |d_axis| = Abs(p_bcast - q) on ScalarE (per-partition bias, exact fp32
      to match the jax reference bit-for-bit), max over 3 axes + fused
      (max<0.1)*(2^22 - pos) score on VectorE. Score rows staged via DRAM.
  Phase 2 (select+gather): per query-tile, top-32 scores = first 32 in-cube
    points via 4 rounds of max8 + match_replace (scores encode position).
    Indices = 2^22 - score; invalid slots -> row 16384 of a zero-padded
    feature table so gathered rows are zero (matches reference masking).
    dma_gather pulls 4096 rows (256B each), PE transposes [128,64]->[64,128]
    per slab, ScalarE copies PSUM->SBUF, DMA writes (q,c,k) output blocks.
"""

import os
import sys

import numpy as np

sys.path.insert(0, "/opt/trn_rl_repo")

B, NB, MB, C = 4, 16384, 4096, 64
NSAMPLE = 32
QPC = 2048  # queries per core
R = float(np.float32(0.1))
BIGPOS = float(4194304.0)  # 2^22
NQT = QPC // 128  # 16 query tiles
NCH = NB // 1024  # 16 point chunks
CHW = 1024  # chunk width

_CACHE = {}
KSTAGE = int(os.environ.get("KSTAGE", "4"))


def _build():
    if "nc" in _CACHE:
        return _CACHE["nc"], _CACHE["names"]
    from contextlib import ExitStack

    import concourse.bass as bass
    import concourse.tile as tile
    from concourse import bacc, mybir

    f32 = mybir.dt.float32
    i32 = mybir.dt.int32
    i16 = mybir.dt.int16
    Alu = mybir.AluOpType
    Act = mybir.ActivationFunctionType

    nc = bacc.Bacc()
    qxyzT = nc.dram_tensor("qxyzT", [3, QPC], f32, kind="ExternalInput")
    pxyzT = nc.dram_tensor("pxyzT", [3, NB], f32, kind="ExternalInput")
    feat = nc.dram_tensor("feat", [NB + 128, C], f32, kind="ExternalInput")
    out_g = nc.dram_tensor("grouped", [QPC, C, NSAMPLE], f32, kind="ExternalOutput")
    out_c = nc.dram_tensor("counts", [QPC], f32, kind="ExternalOutput")

    with tile.TileContext(nc) as tc, ExitStack() as ctx:
        dram = ctx.enter_context(tc.tile_pool(name="dram", bufs=1, space="DRAM"))
        const_p = ctx.enter_context(tc.tile_pool(name="const", bufs=1))
        bc_p = ctx.enter_context(tc.tile_pool(name="bcast", bufs=1))
        pv_p = ctx.enter_context(tc.tile_pool(name="posval", bufs=1))
        abs_p = ctx.enter_context(tc.tile_pool(name="absd", bufs=2))
        sc_p = ctx.enter_context(tc.tile_pool(name="scchunk", bufs=2))
        row_p = ctx.enter_context(tc.tile_pool(name="scorerow", bufs=1))
        sm_p = ctx.enter_context(tc.tile_pool(name="small", bufs=4))
        gat_p = ctx.enter_context(tc.tile_pool(name="gather", bufs=2))
        ob_p = ctx.enter_context(tc.tile_pool(name="outbuf", bufs=3))
        ps_p = ctx.enter_context(tc.tile_pool(name="psum", bufs=2, space="PSUM"))
        psb_p = ctx.enter_context(tc.tile_pool(name="psbc", bufs=1, space="PSUM"))

        # ---- constants ----
        # negated query coords, per axis: [128, NQT] (query q = qt*128 + p)
        qneg = const_p.tile([128, 3 * NQT], f32, tag="qneg")
        for a in range(3):
            qa = sm_p.tile([128, NQT], f32, tag="qld")
            nc.sync.dma_start(
                qa[:], qxyzT[a : a + 1, :].rearrange("o (t p) -> (o p) t", p=128)
            )
            nc.vector.tensor_scalar_mul(qneg[:, a * NQT : (a + 1) * NQT], qa[:], -1.0)

        # ones row for K=1 broadcast matmuls (1.0 is bf16-exact, so the fp32
        # hi/lo weight split reproduces px exactly: out = 1.0*px)
        ones1 = const_p.tile([1, 128], f32, tag="ones1")
        nc.gpsimd.memset(ones1[:], 1.0)

        # identity matrix for PE transpose
        ident = const_p.tile([128, 128], f32, tag="ident")
        nc.gpsimd.memset(ident[:], 1.0)
        nc.gpsimd.affine_select(
            ident[:],
            ident[:],
            pattern=[[1, 128]],
            compare_op=Alu.is_equal,
            fill=0.0,
            base=0,
            channel_multiplier=-1,
        )

        score_d = dram.tile([NQT, 128, NB], f32, tag="scored")
        idx_d = dram.tile([NQT, 128, NSAMPLE], i16, tag="idxd")

        # master position values: 2^22 - j for j in [0, CHW)
        pv0_i = const_p.tile([128, CHW], i32, tag="pv0i")
        nc.gpsimd.iota(
            pv0_i[:], pattern=[[-1, CHW]], base=int(BIGPOS), channel_multiplier=0
        )
        pv0 = const_p.tile([128, CHW], f32, tag="pv0f")
        nc.vector.tensor_copy(pv0[:], pv0_i[:])

        # ---- phase 1: mask + score ----
        for ch in range(NCH):
            pv = pv_p.tile([128, CHW], f32, tag="pvf")
            nc.vector.tensor_scalar_add(pv[:], pv0[:], float(-ch * CHW))

            pb = []
            for a in range(3):
                pr = bc_p.tile([1, CHW], f32, tag=f"pr{a}")
                nc.sync.dma_start(pr[:], pxyzT[a : a + 1, ch * CHW : (ch + 1) * CHW])
                t = psb_p.tile([128, CHW], f32, tag=f"pb{a}")
                for half in range(CHW // 512):
                    nc.tensor.matmul(
                        t[:, half * 512 : (half + 1) * 512],
                        ones1[:],
                        pr[:, half * 512 : (half + 1) * 512],
                        start=True,
                        stop=True,
                    )
                pb.append(t)

            for qt in range(NQT):
                ab = []
                for a in range(3):
                    t = abs_p.tile([128, CHW], f32, tag=f"ab{a}")
                    nc.scalar.activation(
                        t[:],
                        pb[a][:],
                        Act.Abs,
                        bias=qneg[:, a * NQT + qt : a * NQT + qt + 1],
                        scale=1.0,
                    )
                    ab.append(t)
                m2 = sc_p.tile([128, CHW], f32, tag="m2")
                nc.vector.scalar_tensor_tensor(
                    m2[:], ab[0][:], 0.0, ab[1][:], op0=Alu.max, op1=Alu.max
                )
                m3 = sc_p.tile([128, CHW], f32, tag="m3")
                nc.vector.scalar_tensor_tensor(
                    m3[:], ab[2][:], 0.0, m2[:], op0=Alu.max, op1=Alu.max
                )
                sc = sc_p.tile([128, CHW], f32, tag="sc")
                nc.vector.scalar_tensor_tensor(
                    sc[:], m3[:], R, pv[:], op0=Alu.is_lt, op1=Alu.mult
                )
                nc.sync.dma_start(
                    score_d[qt, :, ch * CHW : (ch + 1) * CHW], sc[:]
                )

        # ---- phase 2: select + gather + output ----
        for qt in range(NQT if KSTAGE >= 2 else 0):
            score = row_p.tile([128, NB], f32, tag="score")
            nc.sync.dma_start(score[:], score_d[qt, :, :])

            mx = sm_p.tile([128, NSAMPLE], f32, tag="mx")
            for r in range(4):
                nc.vector.max(mx[:, r * 8 : (r + 1) * 8], score[:])
                if r < 3:
                    nc.vector.match_replace(
                        score[:], mx[:, r * 8 : (r + 1) * 8], score[:], 0.0
                    )

            # idx = min(2^22 - mx, 16384); count = sum(mx > 0)
            idxf = sm_p.tile([128, NSAMPLE], f32, tag="idxf")
            nc.vector.tensor_scalar(
                idxf[:], mx[:], -1.0, BIGPOS, op0=Alu.mult, op1=Alu.add
            )
            idxf2 = sm_p.tile([128, NSAMPLE], f32, tag="idxf2")
            nc.vector.tensor_scalar(idxf2[:], idxf[:], float(NB), None, op0=Alu.min)
            idx16 = sm_p.tile([128, NSAMPLE], i16, tag="idx16")
            nc.vector.tensor_copy(idx16[:], idxf2[:])

            vbit = sm_p.tile([128, NSAMPLE], f32, tag="vbit")
            nc.vector.tensor_scalar(vbit[:], mx[:], 0.0, None, op0=Alu.is_gt)
            cnt = sm_p.tile([128, 1], f32, tag="cnt")
            nc.vector.tensor_reduce(cnt[:], vbit[:], axis=mybir.AxisListType.X, op=Alu.add)
            nc.sync.dma_start(
                out_c[qt * 128 : (qt + 1) * 128].rearrange("(p o) -> p o", o=1), cnt[:]
            )

            # round-trip idx to DRAM, reload 16-wrapped + replicated per core
            nc.sync.dma_start(idx_d[qt, :, :], idx16[:])
            idxw = sm_p.tile([128, 4096 // 16], i16, tag="idxw")
            wrap = idx_d[qt, :, :].rearrange("p k -> (p k)").rearrange(
                "(s w) -> w s", w=16
            )
            for g in range(8):
                nc.sync.dma_start(idxw[g * 16 : (g + 1) * 16, :], wrap)

            if KSTAGE < 3:
                continue
            gath = gat_p.tile([128, NSAMPLE, C], f32, tag="gath")
            for gs in range(4):
                nc.gpsimd.dma_gather(
                    gath[:, gs * 8 : (gs + 1) * 8, :],
                    feat[:, :],
                    idxw[:, gs * 64 : (gs + 1) * 64],
                    num_idxs=1024,
                    num_idxs_reg=1024,
                    elem_size=C,
                )

            # transpose each [128, 64] slab -> [64, 128]; 4 slabs per psum tile
            for grp in range(8 if KSTAGE >= 4 else 0):
                pst = ps_p.tile([64, 512], f32, tag="pst")
                for j in range(4):
                    s = grp * 4 + j
                    nc.tensor.transpose(
                        pst[:, j * 128 : (j + 1) * 128],
                        gath[:, s, :],
                        ident[:],
                    )
                ob = ob_p.tile([64, 512], f32, tag="ob")
                nc.scalar.copy(ob[:], pst[:])
                # queries qt*128 + grp*16 .. +16, layout (c, q, k)
                dst = out_g[
                    qt * 128 + grp * 16 : qt * 128 + (grp + 1) * 16, :, :
                ].rearrange("q c k -> c q k")
                nc.sync.dma_start(
                    dst, ob[:].rearrange("c (q k) -> c q k", k=NSAMPLE)
                )

    nc.finalize()
    names = dict(inputs=["qxyzT", "pxyzT", "feat"], outputs=["grouped", "counts"])
    _CACHE["nc"] = nc
    _CACHE["names"] = names
    return nc, names


def kernel(xyz, xyz_batch_cnt, new_xyz, new_xyz_batch_cnt, features):
    from concourse.bass_utils import run_bass_kernel_spmd

    xyz = np.asarray(xyz, dtype=np.float32)
    new_xyz = np.asarray(new_xyz, dtype=np.float32)
    features = np.asarray(features, dtype=np.float32)

    nc, _ = _build()

    in_maps = []
    for core in range(8):
        b, h = core // 2, core % 2
        q = new_xyz[b * MB + h * QPC : b * MB + (h + 1) * QPC]
        p = xyz[b * NB : (b + 1) * NB]
        f = features[b * NB : (b + 1) * NB]
        faug = np.zeros((NB + 128, C), dtype=np.float32)
        faug[:NB] = f
        in_maps.append(
            {
                "qxyzT": np.ascontiguousarray(q.T),
                "pxyzT": np.ascontiguousarray(p.T),
                "feat": faug,
            }
        )

    res = run_bass_kernel_spmd(nc, in_maps, core_ids=list(range(8)))
    grouped = np.zeros((B * MB, C, NSAMPLE), dtype=np.float32)
    counts = np.zeros((B * MB,), dtype=np.float32)
    for core in range(8):
        b, h = core // 2, core % 2
        lo = b * MB + h * QPC
        r = res.results[core]
        grouped[lo : lo + QPC] = r["grouped"].reshape(QPC, C, NSAMPLE)
        counts[lo : lo + QPC] = r["counts"].reshape(QPC)
    return grouped, counts


if __name__ == "__main__":
    nc, _ = _build()
    print("built ok")
